# revision 4
# baseline (speedup 1.0000x reference)
"""Trainium2 Bass kernel for nn_MAGClassifier (edge-level graph transformer).

Sharding: data-parallel over graphs. 64 graphs / 8 cores = 8 graphs per core.
Each graph: 128 nodes, exactly 512 edges (structured generator), so
to_dense_batch is a reshape and the pad mask is all-True.

Per-graph pipeline (all on device, hid-major layout h^T [128 hid, 512 edges]):
  input MLP via one-hot gather matmuls, 3 transformer layers (masked edge-edge
  attention for 'M' layers via adjacency = M @ M^T > 0, full attention for 'S'),
  PMA pooling, output MLP.

Key device techniques (validated on hardware probes):
  - K=128 matmuls only; per-head QK^T restricts contraction via zero-masked
    per-head q tiles (SBUF->SBUF DMA into persistent zeroed tiles).
  - AV uses lhsT = [V_head | ones] (M=17) col-tiled at 32-pitch partition
    offsets, producing o^T and the softmax denominator Z in one pass.
  - softmax normalization deferred to o^T: recipZ = Exp(-Ln(Z)) (same ACT
    table set), expanded to all partitions with a selection matmul.
  - LayerNorm: mean-centering via CEN = I - J/128 matmul; variance via
    ones-matrix matmul (free partition broadcast); rstd via Exp(-0.5 Ln(v)).
"""
import sys
import os

sys.path.insert(0, "/opt/trn_rl_repo")

import numpy as np
import ml_dtypes

import concourse.bass as bass
import concourse.mybir as mybir
from concourse import bacc
from concourse.tile import TileContext
from concourse.bass_utils import run_bass_kernel_spmd

F32 = mybir.dt.float32
BF16 = mybir.dt.bfloat16
AF = mybir.ActivationFunctionType
OP = mybir.AluOpType

B = 64
NPG = 128
EPG = 512
NODE_DIM = 64
EDGE_DIM = 32
HID = 128
HEADS = 8
DH = 16
L = 3
LAYER_TYPES = ("M", "M", "S")
MLP_HID = 128
N_CORES = 8
GPC = B // N_CORES  # graphs per core
EPS = 1e-5

_BF = ml_dtypes.bfloat16


def _build_program(gpc):
    nc = bacc.Bacc("TRN2", target_bir_lowering=False, debug=False)

    # ---- DRAM inputs (per-core data + replicated weights) ----
    d_xT = nc.dram_tensor("xT", [gpc, 128, 128], F32, kind="ExternalInput")
    d_eaT = nc.dram_tensor("eaT", [gpc, 128, 512], F32, kind="ExternalInput")
    d_ST = nc.dram_tensor("ST", [gpc, 128, 512], F32, kind="ExternalInput")
    d_DT = nc.dram_tensor("DT", [gpc, 128, 512], F32, kind="ExternalInput")

    wnames_f32 = [
        ("W1a", [128, 128]), ("W1b", [128, 128]), ("W1c", [128, 128]),
        ("inb1", [128, 1]), ("inb2", [128, 1]),
        ("CEN", [128, 128]),
        ("outW1", [128, 128]), ("outb1", [128, 1]), ("outW2", [128, 1]),
    ]
    wnames_bf16 = [
        ("onesmat", [128, 128]), ("ident", [128, 128]),
        ("RselA", [128, 128]), ("RselB", [128, 128]),
        ("inW2", [128, 128]), ("qbd", [128, 8]),
        ("pWk", [128, 128]), ("pWv", [128, 128]),
        ("pWoA", [128, 128]), ("pWoB", [128, 128]),
    ]
    for i in range(L):
        for nm in ("WqA", "WqB", "WkA", "WkB", "WvA", "WvB", "WoA", "WoB",
                   "fW1", "fW2"):
            wnames_bf16.append((f"{nm}{i}", [128, 128]))
        for nm in ("ln1g", "ln1b", "ln2g", "ln2b", "fb1", "fb2"):
            wnames_f32.append((f"{nm}{i}", [128, 1]))

    d_w = {}
    for nm, shp in wnames_f32:
        d_w[nm] = nc.dram_tensor(nm, shp, F32, kind="ExternalInput")
    for nm, shp in wnames_bf16:
        d_w[nm] = nc.dram_tensor(nm, shp, BF16, kind="ExternalInput")

    d_out = nc.dram_tensor("out", [1, gpc], F32, kind="ExternalOutput")

    with TileContext(nc) as tc:
        with tc.tile_pool(name="const", bufs=1) as cpool, \
             tc.tile_pool(name="io", bufs=2) as io, \
             tc.tile_pool(name="gr", bufs=2) as gr, \
             tc.tile_pool(name="qkv", bufs=2) as qkv, \
             tc.tile_pool(name="att", bufs=3) as att, \
             tc.tile_pool(name="wk", bufs=2) as wk, \
             tc.tile_pool(name="ps_s", bufs=2, space="PSUM") as ps_s, \
             tc.tile_pool(name="ps_o", bufs=2, space="PSUM") as ps_o, \
             tc.tile_pool(name="ps_m1", bufs=1, space="PSUM") as ps_m1, \
             tc.tile_pool(name="ps_m2", bufs=1, space="PSUM") as ps_m2:

            # ---- load constants ----
            w = {}
            for nm, shp in wnames_f32 + wnames_bf16:
                t = cpool.tile(list(shp), d_w[nm].dtype, tag=f"w_{nm}")
                nc.sync.dma_start(t[:], d_w[nm][:])
                w[nm] = t

            # persistent per-head masked-q tiles (zero rows select the head)
            qTm = []
            for h in range(HEADS):
                t = cpool.tile([128, 512], BF16, tag=f"qTm{h}")
                nc.vector.memset(t[:], 0.0)
                qTm.append(t)

            pooled_acc = cpool.tile([128, gpc], F32, tag="pooled_acc")

            def evac(pool, psum_ap, shape, dtype, tag, engine="v"):
                t = pool.tile(shape, dtype, tag=tag)
                if engine == "v":
                    nc.vector.tensor_copy(t[:], psum_ap)
                else:
                    nc.scalar.activation(t[:], psum_ap, AF.Copy)
                return t

            for g in range(gpc):
                # ---- per-graph loads ----
                xT = io.tile([128, 128], F32, tag="xT")
                nc.sync.dma_start(xT[:], d_xT[g])
                eaT = io.tile([128, 512], F32, tag="eaT")
                nc.sync.dma_start(eaT[:], d_eaT[g])
                ST = io.tile([128, 512], F32, tag="ST")
                nc.sync.dma_start(ST[:], d_ST[g])
                DT = io.tile([128, 512], F32, tag="DT")
                nc.sync.dma_start(DT[:], d_DT[g])

                # ---- adjacency: MT = ST + DT (bf16); C = MT^T @ MT ; adj = min(C,1) ----
                MT = gr.tile([128, 512], BF16, tag="MT")
                nc.vector.tensor_tensor(MT[:], ST[:], DT[:], OP.add)
                adj = gr.tile([128, 2048], BF16, tag="adj")
                for pair in range(2):
                    cps = ps_s.tile([128, 1024], F32, tag="s")
                    for k in range(2):
                        t = 2 * pair + k
                        nc.tensor.matmul(cps[:, 512 * k:512 * (k + 1)],
                                         MT[:, 128 * t:128 * (t + 1)], MT[:],
                                         start=True, stop=True)
                    nc.vector.tensor_scalar(adj[:, 1024 * pair:1024 * (pair + 1)],
                                            cps[:], 1.0, 0.0, OP.min, OP.add)

                # ---- input MLP ----
                ap_ = ps_m1.tile([128, 512], F32, tag="m1")
                nc.tensor.matmul(ap_[:, 0:128], xT[:], w["W1a"][:],
                                 start=True, stop=True)
                nc.tensor.matmul(ap_[:, 128:256], xT[:], w["W1b"][:],
                                 start=True, stop=True)
                AB = evac(wk, ap_[:, 0:256], [128, 256], F32, tag="AB")
                h1p = ps_s.tile([128, 1024], F32, tag="s")
                nc.tensor.matmul(h1p[:, 0:512], AB[:, 0:128], ST[:],
                                 start=True, stop=False)
                nc.tensor.matmul(h1p[:, 0:512], AB[:, 128:256], DT[:],
                                 start=False, stop=False)
                nc.tensor.matmul(h1p[:, 0:512], w["W1c"][:], eaT[:],
                                 start=False, stop=True)
                h1 = wk.tile([128, 512], BF16, tag="h1")
                nc.scalar.activation(h1[:], h1p[:, 0:512], AF.Relu,
                                     bias=w["inb1"][:, 0:1])
                hdp = ps_m1.tile([128, 512], F32, tag="m1")
                nc.tensor.matmul(hdp[:], w["inW2"][:], h1[:], start=True, stop=True)
                hd = wk.tile([128, 512], F32, tag="hd")
                nc.scalar.activation(hd[:], hdp[:], AF.Identity,
                                     bias=w["inb2"][:, 0:1])
                hd_bf = wk.tile([128, 512], BF16, tag="hd_bf")
                nc.vector.tensor_copy(hd_bf[:], hd[:])

                # ---- transformer layers ----
                for li in range(L):
                    masked = LAYER_TYPES[li] == "M"
                    # projections (padded 32-pitch head layout)
                    qp = ps_s.tile([128, 1024], F32, tag="s")
                    nc.tensor.matmul(qp[:, 0:512], w[f"WqA{li}"][:], hd_bf[:],
                                     start=True, stop=True)
                    nc.tensor.matmul(qp[:, 512:1024], w[f"WqB{li}"][:], hd_bf[:],
                                     start=True, stop=True)
                    qT = qkv.tile([128, 1024], BF16, tag="qT")
                    nc.vector.tensor_copy(qT[:], qp[:])
                    kp = ps_s.tile([128, 1024], F32, tag="s")
                    nc.tensor.matmul(kp[:, 0:512], w[f"WkA{li}"][:], hd_bf[:],
                                     start=True, stop=True)
                    nc.tensor.matmul(kp[:, 512:1024], w[f"WkB{li}"][:], hd_bf[:],
                                     start=True, stop=True)
                    kT = qkv.tile([128, 1024], BF16, tag="kT")
                    nc.vector.tensor_copy(kT[:], kp[:])
                    vp = ps_s.tile([128, 1024], F32, tag="s")
                    nc.tensor.matmul(vp[:, 0:512], w[f"WvA{li}"][:], hd_bf[:],
                                     start=True, stop=True)
                    nc.tensor.matmul(vp[:, 512:1024], w[f"WvB{li}"][:], hd_bf[:],
                                     start=True, stop=True)
                    vT = qkv.tile([128, 1024], BF16, tag="vT")
                    nc.vector.tensor_copy(vT[:], vp[:])

                    # masked q head tiles via SBUF->SBUF DMA (zeros persist)
                    for h in range(HEADS):
                        X = 0 if h < 4 else 1
                        hh = h % 4
                        nc.sync.dma_start(
                            qTm[h][32 * hh:32 * hh + 16, :],
                            qT[32 * hh:32 * hh + 16, 512 * X:512 * (X + 1)])

                    # V_aug per chunk: [e2, (head, 17)] from vT transposes
                    vaug = []
                    for c in range(4):
                        vtp = ps_m2.tile([128, 256], BF16, tag="m2")
                        nc.tensor.transpose(vtp[:, 0:128],
                                            vT[:, 0:512][:, 128 * c:128 * (c + 1)],
                                            w["ident"][:])
                        nc.tensor.transpose(vtp[:, 128:256],
                                            vT[:, 512:1024][:, 128 * c:128 * (c + 1)],
                                            w["ident"][:])
                        va = qkv.tile([128, 8, 17], BF16, tag=f"vaug{c}")
                        src = vtp[:].rearrange("p (x h j) -> p (x h) j", x=2, h=4)
                        nc.vector.tensor_copy(va[:, :, 0:16], src[:, :, 0:16])
                        nc.vector.memset(va[:, :, 16:17], 1.0)
                        vaug.append(va)

                    oX = []
                    for _oi in range(2):
                        oXt = ps_o.tile([128, 512], F32, tag="o", name=f"oX{_oi}")
                        oX.append(oXt)
                    for h in range(HEADS):
                        X = 0 if h < 4 else 1
                        hh = h % 4
                        Asb = att.tile([128, 2048], BF16, tag="Asb")
                        for pair in range(2):
                            stile = ps_s.tile([128, 1024], F32, tag="s")
                            for k in range(2):
                                c = 2 * pair + k
                                nc.tensor.matmul(
                                    stile[:, 512 * k:512 * (k + 1)],
                                    kT[:, 512 * X + 128 * c:512 * X + 128 * (c + 1)],
                                    qTm[h][:], start=True, stop=True)
                            nc.scalar.activation(
                                Asb[:, 1024 * pair:1024 * (pair + 1)], stile[:],
                                AF.Exp, scale=0.25)
                        if masked:
                            nc.vector.tensor_tensor(Asb[:], Asb[:], adj[:], OP.mult)
                        for c in range(4):
                            nc.tensor.matmul(
                                oX[X][32 * hh:32 * hh + 17, :],
                                vaug[c][:, h, :],
                                Asb[:, 512 * c:512 * (c + 1)],
                                start=(c == 0), stop=(c == 3),
                                tile_position=(0, 32 * hh))

                    # recipZ + o-normalization + Wo
                    owp = ps_m1.tile([128, 512], F32, tag="m1")
                    for X in range(2):
                        zs = wk.tile([128, 512], F32, tag="zs")
                        nc.vector.tensor_scalar_max(zs[:], oX[X][:], 1e-20)
                        lz = wk.tile([128, 512], F32, tag="lz")
                        nc.scalar.activation(lz[:], zs[:], AF.Ln)
                        R = wk.tile([128, 512], BF16, tag="R")
                        nc.scalar.activation(R[:], lz[:], AF.Exp, scale=-1.0)
                        rbp = ps_m2.tile([128, 512], F32, tag="m2")
                        nc.tensor.matmul(rbp[:], w["RselA" if X == 0 else "RselB"][:],
                                         R[:], start=True, stop=True)
                        Rb = wk.tile([128, 512], F32, tag="Rb")
                        nc.vector.tensor_copy(Rb[:], rbp[:])
                        oT = wk.tile([128, 512], BF16, tag=f"oT{X}")
                        nc.vector.tensor_tensor(oT[:], oX[X][:], Rb[:], OP.mult)
                        nc.tensor.matmul(owp[:], w[("WoA" if X == 0 else "WoB") + str(li)][:],
                                         oT[:], start=(X == 0), stop=(X == 1))

                    u = wk.tile([128, 512], F32, tag="u")
                    nc.vector.tensor_tensor(u[:], owp[:], hd[:], OP.add)

                    # LN1
                    ucp = ps_m1.tile([128, 512], F32, tag="m1")
                    nc.tensor.matmul(ucp[:], w["CEN"][:], u[:], start=True, stop=True)
                    uc = evac(wk, ucp[:], [128, 512], F32, tag="uc")
                    sq = wk.tile([128, 512], BF16, tag="sq")
                    nc.vector.tensor_tensor(sq[:], uc[:], uc[:], OP.mult)
                    ssp = ps_m1.tile([128, 512], F32, tag="m1")
                    nc.tensor.matmul(ssp[:], w["onesmat"][:], sq[:],
                                     start=True, stop=True)
                    vb = wk.tile([128, 512], F32, tag="vb")
                    nc.vector.tensor_scalar(vb[:], ssp[:], 1.0 / 128.0, EPS,
                                            OP.mult, OP.add)
                    lnv = wk.tile([128, 512], F32, tag="lnv")
                    nc.scalar.activation(lnv[:], vb[:], AF.Ln)
                    rb2 = wk.tile([128, 512], F32, tag="rb2")
                    nc.scalar.activation(rb2[:], lnv[:], AF.Exp, scale=-0.5)
                    t2 = wk.tile([128, 512], F32, tag="t2")
                    nc.vector.tensor_tensor(t2[:], uc[:], rb2[:], OP.mult)
                    hd1 = wk.tile([128, 512], F32, tag="hd1")
                    nc.scalar.activation(hd1[:], t2[:], AF.Identity,
                                         scale=w[f"ln1g{li}"][:, 0:1],
                                         bias=w[f"ln1b{li}"][:, 0:1])
                    hd1_bf = wk.tile([128, 512], BF16, tag="hd1_bf")
                    nc.vector.tensor_copy(hd1_bf[:], hd1[:])

                    # FFN
                    f1p = ps_m1.tile([128, 512], F32, tag="m1")
                    nc.tensor.matmul(f1p[:], w[f"fW1{li}"][:], hd1_bf[:],
                                     start=True, stop=True)
                    f1 = wk.tile([128, 512], BF16, tag="f1")
                    nc.scalar.activation(f1[:], f1p[:], AF.Relu,
                                         bias=w[f"fb1{li}"][:, 0:1])
                    f2p = ps_m1.tile([128, 512], F32, tag="m1")
                    nc.tensor.matmul(f2p[:], w[f"fW2{li}"][:], f1[:],
                                     start=True, stop=True)
                    f2b = wk.tile([128, 512], F32, tag="f2b")
                    nc.scalar.activation(f2b[:], f2p[:], AF.Identity,
                                         bias=w[f"fb2{li}"][:, 0:1])
                    u2 = wk.tile([128, 512], F32, tag="u2")
                    nc.vector.tensor_tensor(u2[:], f2b[:], hd1[:], OP.add)

                    # LN2
                    ucp2 = ps_m1.tile([128, 512], F32, tag="m1")
                    nc.tensor.matmul(ucp2[:], w["CEN"][:], u2[:], start=True, stop=True)
                    uc2 = evac(wk, ucp2[:], [128, 512], F32, tag="uc")
                    sq2 = wk.tile([128, 512], BF16, tag="sq")
                    nc.vector.tensor_tensor(sq2[:], uc2[:], uc2[:], OP.mult)
                    ssp2 = ps_m1.tile([128, 512], F32, tag="m1")
                    nc.tensor.matmul(ssp2[:], w["onesmat"][:], sq2[:],
                                     start=True, stop=True)
                    vb2 = wk.tile([128, 512], F32, tag="vb")
                    nc.vector.tensor_scalar(vb2[:], ssp2[:], 1.0 / 128.0, EPS,
                                            OP.mult, OP.add)
                    lnv2 = wk.tile([128, 512], F32, tag="lnv")
                    nc.scalar.activation(lnv2[:], vb2[:], AF.Ln)
                    rb3 = wk.tile([128, 512], F32, tag="rb2")
                    nc.scalar.activation(rb3[:], lnv2[:], AF.Exp, scale=-0.5)
                    t3 = wk.tile([128, 512], F32, tag="t2")
                    nc.vector.tensor_tensor(t3[:], uc2[:], rb3[:], OP.mult)
                    hd = wk.tile([128, 512], F32, tag="hd")
                    nc.scalar.activation(hd[:], t3[:], AF.Identity,
                                         scale=w[f"ln2g{li}"][:, 0:1],
                                         bias=w[f"ln2b{li}"][:, 0:1])
                    hd_bf = wk.tile([128, 512], BF16, tag="hd_bf")
                    nc.vector.tensor_copy(hd_bf[:], hd[:])

                # ---- PMA pooling ----
                kpp = ps_m1.tile([128, 512], F32, tag="m1")
                nc.tensor.matmul(kpp[:], w["pWk"][:], hd_bf[:], start=True, stop=True)
                kpb = wk.tile([128, 512], BF16, tag="kpb")
                nc.vector.tensor_copy(kpb[:], kpp[:])
                vpp = ps_m1.tile([128, 512], F32, tag="m1")
                nc.tensor.matmul(vpp[:], w["pWv"][:], hd_bf[:], start=True, stop=True)
                vpb = wk.tile([128, 512], BF16, tag="vpb")
                nc.vector.tensor_copy(vpb[:], vpp[:])

                spp = ps_m1.tile([128, 512], F32, tag="m1")
                nc.tensor.matmul(spp[0:8, :], w["qbd"][:], kpb[:],
                                 start=True, stop=True)
                esb = wk.tile([8, 512], F32, tag="esb")
                nc.scalar.activation(esb[:], spp[0:8, :], AF.Exp, scale=0.25)
                zp = wk.tile([8, 1], F32, tag="zp")
                nc.vector.tensor_reduce(zp[:], esb[:], mybir.AxisListType.X, OP.add)
                rz = wk.tile([8, 1], F32, tag="rz")
                nc.vector.reciprocal(rz[:], zp[:])
                asb = wk.tile([128, 512], BF16, tag="asb")
                nc.vector.memset(asb[:], 0.0)
                nc.vector.tensor_tensor(asb[0:8, :], esb[:],
                                        rz[:].to_broadcast((8, 512)), OP.mult)

                # transpose a [8,512] (padded to 128 rows) and Vp
                atp = ps_m2.tile([128, 512], BF16, tag="m2")
                for c in range(4):
                    nc.tensor.transpose(atp[:, 128 * c:128 * (c + 1)],
                                        asb[:, 128 * c:128 * (c + 1)],
                                        w["ident"][:])
                aT = wk.tile([128, 512], BF16, tag="aT")
                nc.vector.tensor_copy(aT[:], atp[:])
                vtp2 = ps_m2.tile([128, 512], BF16, tag="m2")
                for c in range(4):
                    nc.tensor.transpose(vtp2[:, 128 * c:128 * (c + 1)],
                                        vpb[:, 128 * c:128 * (c + 1)],
                                        w["ident"][:])
                Vp = wk.tile([128, 512], BF16, tag="Vp")
                nc.vector.tensor_copy(Vp[:], vtp2[:])

                opX = []
                for _oi in range(2):
                    opXt = ps_o.tile([128, 512], F32, tag="o", name=f"opX{_oi}")
                    opX.append(opXt)
                for h in range(HEADS):
                    X = 0 if h < 4 else 1
                    hh = h % 4
                    for c in range(4):
                        nc.tensor.matmul(
                            opX[X][32 * hh:32 * hh + 16, 0:1],
                            Vp[:, 128 * c + 16 * h:128 * c + 16 * h + 16],
                            aT[:, 128 * c + h:128 * c + h + 1],
                            start=(c == 0), stop=(c == 3),
                            tile_position=(0, 32 * hh))
                opsb = [None, None]
                for X in range(2):
                    t = wk.tile([128, 16], BF16, tag=f"opsb{X}")
                    nc.vector.tensor_copy(t[:], opX[X][:, 0:16])
                    opsb[X] = t
                plp = ps_m1.tile([128, 512], F32, tag="m1")
                nc.tensor.matmul(plp[:, 0:1], w["pWoA"][:], opsb[0][:, 0:1],
                                 start=True, stop=False)
                nc.tensor.matmul(plp[:, 0:1], w["pWoB"][:], opsb[1][:, 0:1],
                                 start=False, stop=True)
                nc.vector.tensor_copy(pooled_acc[:, g:g + 1], plp[:, 0:1])

            # ---- output MLP over all graphs of this core ----
            t1p = ps_m1.tile([128, 512], F32, tag="m1")
            nc.tensor.matmul(t1p[:, 0:gpc], w["outW1"][:], pooled_acc[:],
                             start=True, stop=True)
            t1 = wk.tile([128, gpc], F32, tag="t1")
            nc.scalar.activation(t1[:], t1p[:, 0:gpc], AF.Relu,
                                 bias=w["outb1"][:, 0:1])
            t2p = ps_m1.tile([128, 512], F32, tag="m1")
            nc.tensor.matmul(t2p[0:1, 0:gpc], w["outW2"][:], t1[:],
                             start=True, stop=True)
            outsb = wk.tile([1, gpc], F32, tag="outsb")
            nc.vector.tensor_copy(outsb[:], t2p[0:1, 0:gpc])
            nc.sync.dma_start(d_out[:], outsb[:])

    nc.compile()
    return nc


_CACHED = {}


def _get_program(gpc):
    if gpc not in _CACHED:
        _CACHED[gpc] = _build_program(gpc)
    return _CACHED[gpc]


def _prep_weights(kw):
    """Host-side packing of weights into the layouts the device expects."""
    f = np.asarray
    out = {}
    in_W1 = f(kw["in_W1"], np.float32)  # [160, 128]
    W1a = np.zeros((128, 128), np.float32); W1a[:64] = in_W1[0:64]
    W1b = np.zeros((128, 128), np.float32); W1b[:64] = in_W1[64:128]
    W1c = np.zeros((128, 128), np.float32); W1c[:32] = in_W1[128:160]
    out["W1a"], out["W1b"], out["W1c"] = W1a, W1b, W1c
    out["inb1"] = f(kw["in_b1"], np.float32).reshape(128, 1)
    out["inb2"] = f(kw["in_b2"], np.float32).reshape(128, 1)
    out["inW2"] = f(kw["in_W2"], np.float32).astype(_BF)
    out["CEN"] = (np.eye(128) - np.ones((128, 128)) / 128.0).astype(np.float32)
    out["onesmat"] = np.ones((128, 128), _BF)
    out["ident"] = np.eye(128, dtype=_BF)
    # Rsel: Rbig rows 32h+j (j<16) <- R row 32h+16
    RselA = np.zeros((128, 128), np.float32)
    for hh in range(4):
        RselA[32 * hh + 16, 32 * hh:32 * hh + 16] = 1.0
    out["RselA"] = RselA.astype(_BF)
    out["RselB"] = RselA.astype(_BF)

    def pad_heads(W):
        # W [128, 128] -> (A, B): A cols 32hh+j = W[:, (hh)*16+j], h<4
        A = np.zeros((128, 128), np.float32)
        Bm = np.zeros((128, 128), np.float32)
        for hh in range(4):
            A[:, 32 * hh:32 * hh + 16] = W[:, 16 * hh:16 * hh + 16]
            Bm[:, 32 * hh:32 * hh + 16] = W[:, 16 * (hh + 4):16 * (hh + 4) + 16]
        return A.astype(_BF), Bm.astype(_BF)

    def pad_heads_rows(W):
        # W [128, 128] -> (A, B) with PADDED ROWS: A rows 32hh+j = W[16hh+j,:]
        A = np.zeros((128, 128), np.float32)
        Bm = np.zeros((128, 128), np.float32)
        for hh in range(4):
            A[32 * hh:32 * hh + 16, :] = W[16 * hh:16 * hh + 16, :]
            Bm[32 * hh:32 * hh + 16, :] = W[16 * (hh + 4):16 * (hh + 4) + 16, :]
        return A.astype(_BF), Bm.astype(_BF)

    Wq, Wk, Wv, Wo = (f(kw[n], np.float32) for n in ("Wq", "Wk", "Wv", "Wo"))
    for i in range(L):
        out[f"WqA{i}"], out[f"WqB{i}"] = pad_heads(Wq[i])
        out[f"WkA{i}"], out[f"WkB{i}"] = pad_heads(Wk[i])
        out[f"WvA{i}"], out[f"WvB{i}"] = pad_heads(Wv[i])
        out[f"WoA{i}"], out[f"WoB{i}"] = pad_heads_rows(Wo[i])
        out[f"fW1{i}"] = f(kw["f_W1"][i], np.float32).astype(_BF)
        out[f"fW2{i}"] = f(kw["f_W2"][i], np.float32).astype(_BF)
        out[f"fb1{i}"] = f(kw["f_b1"][i], np.float32).reshape(128, 1)
        out[f"fb2{i}"] = f(kw["f_b2"][i], np.float32).reshape(128, 1)
        out[f"ln1g{i}"] = f(kw["ln1_g"][i], np.float32).reshape(128, 1)
        out[f"ln1b{i}"] = f(kw["ln1_b"][i], np.float32).reshape(128, 1)
        out[f"ln2g{i}"] = f(kw["ln2_g"][i], np.float32).reshape(128, 1)
        out[f"ln2b{i}"] = f(kw["ln2_b"][i], np.float32).reshape(128, 1)

    # PMA
    seed = f(kw["seed"], np.float32)
    qp = seed @ f(kw["p_Wq"], np.float32)  # [128]
    qbd = np.zeros((128, 8), np.float32)
    for h in range(HEADS):
        qbd[16 * h:16 * h + 16, h] = qp[16 * h:16 * h + 16]
    out["qbd"] = qbd.astype(_BF)
    out["pWk"] = f(kw["p_Wk"], np.float32).astype(_BF)
    out["pWv"] = f(kw["p_Wv"], np.float32).astype(_BF)
    pWo = f(kw["p_Wo"], np.float32)
    # pWoA rows 32hh+j <-> p_Wo[(hh)*16+j, :] (heads 0-3); B heads 4-7
    A = np.zeros((128, 128), np.float32)
    Bm = np.zeros((128, 128), np.float32)
    for hh in range(4):
        A[32 * hh:32 * hh + 16, :] = pWo[16 * hh:16 * hh + 16, :]
        Bm[32 * hh:32 * hh + 16, :] = pWo[16 * (hh + 4):16 * (hh + 4) + 16, :]
    out["pWoA"] = A.astype(_BF)
    out["pWoB"] = Bm.astype(_BF)
    out["outW1"] = f(kw["out_W1"], np.float32)
    out["outb1"] = f(kw["out_b1"], np.float32).reshape(128, 1)
    out["outW2"] = f(kw["out_W2"], np.float32).reshape(128, 1)
    return out, float(np.asarray(kw["out_b2"]).reshape(-1)[0])


def _reference_numpy(kw):
    """Exact numpy fallback for unstructured inputs."""
    x = np.asarray(kw["x"], np.float32)
    ea = np.asarray(kw["edge_attr"], np.float32)
    src = np.asarray(kw["edge_index"])[0].astype(np.int64)
    dst = np.asarray(kw["edge_index"])[1].astype(np.int64)
    nb = np.asarray(kw["node_batch"]).astype(np.int64)
    ME = int(np.asarray(kw["max_edges"]))
    E = src.shape[0]

    def ln(x_, g_, b_):
        m = x_.mean(-1, keepdims=True)
        v = x_.var(-1, keepdims=True)
        return (x_ - m) / np.sqrt(v + 1e-5) * g_ + b_

    def mha(q_in, kv_in, Wq_, Wk_, Wv_, Wo_, mask):
        Bq, Lq, _ = q_in.shape
        Lk = kv_in.shape[1]
        q = (q_in @ Wq_).reshape(Bq, Lq, HEADS, DH)
        k = (kv_in @ Wk_).reshape(Bq, Lk, HEADS, DH)
        v = (kv_in @ Wv_).reshape(Bq, Lk, HEADS, DH)
        s = np.einsum('bqhd,bkhd->bhqk', q, k) / np.sqrt(DH)
        s = np.where(mask[:, None, :, :], s, -1e9).astype(np.float32)
        s = s - s.max(-1, keepdims=True)
        a = np.exp(s); a /= a.sum(-1, keepdims=True)
        o = np.einsum('bhqk,bkhd->bqhd', a, v).reshape(Bq, Lq, HID)
        return o @ Wo_

    feat = np.concatenate([x[src], x[dst], ea], 1)
    h = np.maximum(feat @ kw["in_W1"] + kw["in_b1"], 0) @ kw["in_W2"] + kw["in_b2"]
    eb = nb[src]
    counts = np.bincount(eb, minlength=B)
    offsets = np.concatenate([[0], np.cumsum(counts)[:-1]])
    pos = np.arange(E) - offsets[eb]
    dense_h = np.zeros((B, ME, HID), np.float32)
    dense_h[eb, pos] = h
    pad = np.zeros((B, ME), bool); pad[eb, pos] = True
    ds_ = np.full((B, ME), -1, np.int64); ds_[eb, pos] = src
    dd_ = np.full((B, ME), -2, np.int64); dd_[eb, pos] = dst
    adj = ((ds_[:, :, None] == ds_[:, None, :]) | (ds_[:, :, None] == dd_[:, None, :]) |
           (dd_[:, :, None] == ds_[:, None, :]) | (dd_[:, :, None] == dd_[:, None, :]))
    eye = np.eye(ME, dtype=bool)[None]
    adj_mask = (adj & pad[:, None, :]) | eye
    pad2 = np.broadcast_to(pad[:, None, :], (B, ME, ME)) | eye
    hd = dense_h
    for i, t in enumerate(LAYER_TYPES):
        m = adj_mask if t == 'M' else pad2
        a = mha(hd, hd, kw["Wq"][i], kw["Wk"][i], kw["Wv"][i], kw["Wo"][i], m)
        hd = ln(hd + a, kw["ln1_g"][i], kw["ln1_b"][i])
        fv = np.maximum(hd @ kw["f_W1"][i] + kw["f_b1"][i], 0) @ kw["f_W2"][i] + kw["f_b2"][i]
        hd = ln(hd + fv, kw["ln2_g"][i], kw["ln2_b"][i])
    q = np.broadcast_to(np.asarray(kw["seed"], np.float32)[None, None, :], (B, 1, HID))
    pooled = mha(q, hd, kw["p_Wq"], kw["p_Wk"], kw["p_Wv"], kw["p_Wo"],
                 pad[:, None, :])[:, 0]
    logits = np.maximum(pooled @ kw["out_W1"] + kw["out_b1"], 0) @ kw["out_W2"] + kw["out_b2"]
    return logits.reshape(-1).astype(np.float32)


def kernel(**inputs):
    kw = {k: np.asarray(v) for k, v in inputs.items()}
    src = kw["edge_index"][0].astype(np.int64)
    dst = kw["edge_index"][1].astype(np.int64)
    nb = kw["node_batch"].astype(np.int64)
    ME = int(np.asarray(kw["max_edges"]))

    # structured-input check: 128 nodes/graph sorted, 512 edges/graph in order
    structured = (
        ME == EPG
        and nb.shape[0] == B * NPG
        and np.array_equal(nb, np.repeat(np.arange(B), NPG))
        and src.shape[0] == B * EPG
        and np.array_equal(src // NPG, np.repeat(np.arange(B), EPG))
        and np.array_equal(dst // NPG, np.repeat(np.arange(B), EPG))
    )
    if not structured:
        return _reference_numpy(kw)

    x = kw["x"].astype(np.float32)          # [8192, 64]
    ea = kw["edge_attr"].astype(np.float32)  # [32768, 32]
    srcl = (src % NPG).reshape(B, EPG)
    dstl = (dst % NPG).reshape(B, EPG)

    # host layout prep
    xT = np.zeros((B, 128, 128), np.float32)
    xT[:, 0:64, :] = x.reshape(B, NPG, NODE_DIM).transpose(0, 2, 1)
    eaT = np.zeros((B, 128, 512), np.float32)
    eaT[:, 0:32, :] = ea.reshape(B, EPG, EDGE_DIM).transpose(0, 2, 1)
    # one-hots [node, edge]
    ST = np.zeros((B, 128, 512), np.float32)
    DT = np.zeros((B, 128, 512), np.float32)
    bidx = np.repeat(np.arange(B), EPG)
    eidx = np.tile(np.arange(EPG), B)
    ST[bidx, srcl.reshape(-1), eidx] = 1.0
    DT[bidx, dstl.reshape(-1), eidx] = 1.0

    wmap, out_b2 = _prep_weights(kw)
    nc = _get_program(GPC)

    in_maps = []
    for c in range(N_CORES):
        sl = slice(c * GPC, (c + 1) * GPC)
        m = dict(xT=np.ascontiguousarray(xT[sl]),
                 eaT=np.ascontiguousarray(eaT[sl]),
                 ST=np.ascontiguousarray(ST[sl]),
                 DT=np.ascontiguousarray(DT[sl]))
        for k2, v2 in wmap.items():
            m[k2] = v2
        in_maps.append(m)

    res = run_bass_kernel_spmd(nc, in_maps, list(range(N_CORES)))
    outs = [np.asarray(res.results[c]["out"], np.float32).reshape(-1)
            for c in range(N_CORES)]
    logits = np.concatenate(outs) + out_b2
    return logits.astype(np.float32)


if __name__ == "__main__":
    pass


# revision 5
# speedup vs baseline: 1.7743x; 1.7743x over previous
"""Trainium2 Bass kernel for nn_MAGClassifier (edge-level graph transformer).

Sharding: data-parallel over graphs. 64 graphs / 8 cores = 8 graphs per core.
Each graph: 128 nodes, exactly 512 edges (structured generator), so
to_dense_batch is a reshape and the pad mask is all-True.

Per-graph pipeline (all on device, hid-major layout h^T [128 hid, 512 edges]):
  input MLP via one-hot gather matmuls, 3 transformer layers (masked edge-edge
  attention for 'M' layers via adjacency = M @ M^T > 0, full attention for 'S'),
  PMA pooling, output MLP.

Key device techniques (validated on hardware probes):
  - K=128 matmuls only; per-head QK^T restricts contraction via zero-masked
    per-head q tiles (SBUF->SBUF DMA into persistent zeroed tiles).
  - AV uses lhsT = [V_head | ones] (M=17) col-tiled at 32-pitch partition
    offsets, producing o^T and the softmax denominator Z in one pass.
  - softmax normalization deferred to o^T: recipZ = Exp(-Ln(Z)) (same ACT
    table set), expanded to all partitions with a selection matmul.
  - LayerNorm: mean-centering via CEN = I - J/128 matmul; variance via
    ones-matrix matmul (free partition broadcast); rstd via Exp(-0.5 Ln(v)).
"""
import sys
import os

sys.path.insert(0, "/opt/trn_rl_repo")

import numpy as np
import ml_dtypes

import concourse.bass as bass
import concourse.mybir as mybir
from concourse import bacc
from concourse.tile import TileContext
from concourse.bass_utils import run_bass_kernel_spmd

F32 = mybir.dt.float32
BF16 = mybir.dt.bfloat16
AF = mybir.ActivationFunctionType
OP = mybir.AluOpType

B = 64
NPG = 128
EPG = 512
NODE_DIM = 64
EDGE_DIM = 32
HID = 128
HEADS = 8
DH = 16
L = 3
LAYER_TYPES = ("M", "M", "S")
MLP_HID = 128
N_CORES = 8
GPC = B // N_CORES  # graphs per core
EPS = 1e-5

_BF = ml_dtypes.bfloat16


def _build_program(gpc):
    nc = bacc.Bacc("TRN2", target_bir_lowering=False, debug=False)

    # ---- DRAM inputs (per-core data + replicated weights) ----
    d_xT = nc.dram_tensor("xT", [gpc, 64, 128], F32, kind="ExternalInput")
    d_eaT = nc.dram_tensor("eaT", [gpc, 32, 512], F32, kind="ExternalInput")
    d_src = nc.dram_tensor("srcb", [gpc, 1, 512], F32, kind="ExternalInput")
    d_dst = nc.dram_tensor("dstb", [gpc, 1, 512], F32, kind="ExternalInput")

    wnames_f32 = [
        ("W1a", [128, 128]), ("W1b", [128, 128]), ("W1c", [128, 128]),
        ("inb1", [128, 1]), ("inb2", [128, 1]),
        ("CEN", [128, 128]), ("iota", [128, 1]),
        ("outW1", [128, 128]), ("outb1", [128, 1]), ("outW2", [128, 1]),
    ]
    wnames_bf16 = [
        ("onesmat", [128, 128]), ("ident", [128, 128]),
        ("RselA", [128, 128]), ("RselB", [128, 128]),
        ("inW2", [128, 128]), ("qbd", [128, 8]),
        ("pWk", [128, 128]), ("pWv", [128, 128]),
        ("pWoA", [128, 128]), ("pWoB", [128, 128]),
    ]
    for i in range(L):
        for nm in ("WqA", "WqB", "WkA", "WkB", "WvA", "WvB", "WoA", "WoB",
                   "fW1", "fW2"):
            wnames_bf16.append((f"{nm}{i}", [128, 128]))
        for nm in ("ln1g", "ln1b", "ln2g", "ln2b", "fb1", "fb2"):
            wnames_f32.append((f"{nm}{i}", [128, 1]))

    d_w = {}
    for nm, shp in wnames_f32:
        d_w[nm] = nc.dram_tensor(nm, shp, F32, kind="ExternalInput")
    for nm, shp in wnames_bf16:
        d_w[nm] = nc.dram_tensor(nm, shp, BF16, kind="ExternalInput")

    d_out = nc.dram_tensor("out", [1, gpc], F32, kind="ExternalOutput")

    with TileContext(nc) as tc:
        with tc.tile_pool(name="const", bufs=1) as cpool, \
             tc.tile_pool(name="io", bufs=2) as io, \
             tc.tile_pool(name="gr", bufs=2) as gr, \
             tc.tile_pool(name="qkv", bufs=2) as qkv, \
             tc.tile_pool(name="att", bufs=3) as att, \
             tc.tile_pool(name="wk", bufs=2) as wk, \
             tc.tile_pool(name="ps_s", bufs=2, space="PSUM") as ps_s, \
             tc.tile_pool(name="ps_o", bufs=2, space="PSUM") as ps_o, \
             tc.tile_pool(name="ps_m1", bufs=1, space="PSUM") as ps_m1, \
             tc.tile_pool(name="ps_m2", bufs=1, space="PSUM") as ps_m2:

            # ---- load constants ----
            w = {}
            for nm, shp in wnames_f32 + wnames_bf16:
                t = cpool.tile(list(shp), d_w[nm].dtype, tag=f"w_{nm}")
                nc.sync.dma_start(t[:], d_w[nm][:])
                w[nm] = t

            # persistent per-head masked-q tiles (zero rows select the head)
            qTm = []
            for h in range(HEADS):
                t = cpool.tile([128, 512], BF16, tag=f"qTm{h}")
                nc.vector.memset(t[:], 0.0)
                qTm.append(t)

            pooled_acc = cpool.tile([128, gpc], F32, tag="pooled_acc")

            xT = cpool.tile([128, 128], F32, tag="xT_t")
            nc.vector.memset(xT[:], 0.0)
            eaT = cpool.tile([128, 512], F32, tag="eaT_t")
            nc.vector.memset(eaT[:], 0.0)

            def evac(pool, psum_ap, shape, dtype, tag, engine="v"):
                t = pool.tile(shape, dtype, tag=tag)
                if engine == "v":
                    nc.vector.tensor_copy(t[:], psum_ap)
                else:
                    nc.scalar.activation(t[:], psum_ap, AF.Copy)
                return t

            for g in range(gpc):
                # ---- per-graph loads ----
                nc.sync.dma_start(xT[0:64, :], d_xT[g])
                nc.sync.dma_start(eaT[0:32, :], d_eaT[g])
                srcr = io.tile([128, 512], F32, tag="srcr")
                sap = d_src[g]
                nc.gpsimd.dma_start(
                    out=srcr[:],
                    in_=bass.AP(tensor=sap.tensor, offset=sap.offset,
                                ap=[[0, 128]] + list(sap.ap[1:])))
                dstr = io.tile([128, 512], F32, tag="dstr")
                dap = d_dst[g]
                nc.gpsimd.dma_start(
                    out=dstr[:],
                    in_=bass.AP(tensor=dap.tensor, offset=dap.offset,
                                ap=[[0, 128]] + list(dap.ap[1:])))
                ST = io.tile([128, 512], F32, tag="ST")
                nc.vector.tensor_tensor(ST[:], srcr[:],
                                        w["iota"][:, 0:1].to_broadcast((128, 512)),
                                        OP.is_equal)
                DT = io.tile([128, 512], F32, tag="DT")
                nc.vector.tensor_tensor(DT[:], dstr[:],
                                        w["iota"][:, 0:1].to_broadcast((128, 512)),
                                        OP.is_equal)

                # ---- adjacency: MT = ST + DT (bf16); C = MT^T @ MT ; adj = min(C,1) ----
                MT = gr.tile([128, 512], BF16, tag="MT")
                nc.vector.tensor_tensor(MT[:], ST[:], DT[:], OP.add)
                adj = gr.tile([128, 2048], BF16, tag="adj")
                for pair in range(2):
                    cps = ps_s.tile([128, 1024], F32, tag="s")
                    for k in range(2):
                        t = 2 * pair + k
                        nc.tensor.matmul(cps[:, 512 * k:512 * (k + 1)],
                                         MT[:, 128 * t:128 * (t + 1)], MT[:],
                                         start=True, stop=True)
                    nc.vector.tensor_scalar(adj[:, 1024 * pair:1024 * (pair + 1)],
                                            cps[:], 1.0, 0.0, OP.min, OP.add)

                # ---- input MLP ----
                ap_ = ps_m1.tile([128, 512], F32, tag="m1")
                nc.tensor.matmul(ap_[:, 0:128], xT[:], w["W1a"][:],
                                 start=True, stop=True)
                nc.tensor.matmul(ap_[:, 128:256], xT[:], w["W1b"][:],
                                 start=True, stop=True)
                AB = evac(wk, ap_[:, 0:256], [128, 256], F32, tag="AB")
                h1p = ps_s.tile([128, 1024], F32, tag="s")
                nc.tensor.matmul(h1p[:, 0:512], AB[:, 0:128], ST[:],
                                 start=True, stop=False)
                nc.tensor.matmul(h1p[:, 0:512], AB[:, 128:256], DT[:],
                                 start=False, stop=False)
                nc.tensor.matmul(h1p[:, 0:512], w["W1c"][:], eaT[:],
                                 start=False, stop=True)
                h1 = wk.tile([128, 512], BF16, tag="h1")
                nc.scalar.activation(h1[:], h1p[:, 0:512], AF.Relu,
                                     bias=w["inb1"][:, 0:1])
                hdp = ps_m1.tile([128, 512], F32, tag="m1")
                nc.tensor.matmul(hdp[:], w["inW2"][:], h1[:], start=True, stop=True)
                hd = wk.tile([128, 512], F32, tag="hd")
                nc.scalar.activation(hd[:], hdp[:], AF.Identity,
                                     bias=w["inb2"][:, 0:1])
                hd_bf = wk.tile([128, 512], BF16, tag="hd_bf")
                nc.vector.tensor_copy(hd_bf[:], hd[:])

                # ---- transformer layers ----
                for li in range(L):
                    masked = LAYER_TYPES[li] == "M"
                    # projections (padded 32-pitch head layout)
                    qp = ps_s.tile([128, 1024], F32, tag="s")
                    nc.tensor.matmul(qp[:, 0:512], w[f"WqA{li}"][:], hd_bf[:],
                                     start=True, stop=True)
                    nc.tensor.matmul(qp[:, 512:1024], w[f"WqB{li}"][:], hd_bf[:],
                                     start=True, stop=True)
                    qT = qkv.tile([128, 1024], BF16, tag="qT")
                    nc.vector.tensor_copy(qT[:], qp[:])
                    kp = ps_s.tile([128, 1024], F32, tag="s")
                    nc.tensor.matmul(kp[:, 0:512], w[f"WkA{li}"][:], hd_bf[:],
                                     start=True, stop=True)
                    nc.tensor.matmul(kp[:, 512:1024], w[f"WkB{li}"][:], hd_bf[:],
                                     start=True, stop=True)
                    kT = qkv.tile([128, 1024], BF16, tag="kT")
                    nc.vector.tensor_copy(kT[:], kp[:])
                    vp = ps_s.tile([128, 1024], F32, tag="s")
                    nc.tensor.matmul(vp[:, 0:512], w[f"WvA{li}"][:], hd_bf[:],
                                     start=True, stop=True)
                    nc.tensor.matmul(vp[:, 512:1024], w[f"WvB{li}"][:], hd_bf[:],
                                     start=True, stop=True)
                    vT = qkv.tile([128, 1024], BF16, tag="vT")
                    nc.vector.tensor_copy(vT[:], vp[:])

                    # masked q head tiles via SBUF->SBUF DMA (zeros persist)
                    for h in range(HEADS):
                        X = 0 if h < 4 else 1
                        hh = h % 4
                        nc.sync.dma_start(
                            qTm[h][32 * hh:32 * hh + 16, :],
                            qT[32 * hh:32 * hh + 16, 512 * X:512 * (X + 1)])

                    # V_aug per chunk: [e2, (head, 17)] from vT transposes
                    vaug = []
                    for c in range(4):
                        vtp = ps_m2.tile([128, 256], BF16, tag="m2")
                        nc.tensor.transpose(vtp[:, 0:128],
                                            vT[:, 0:512][:, 128 * c:128 * (c + 1)],
                                            w["ident"][:])
                        nc.tensor.transpose(vtp[:, 128:256],
                                            vT[:, 512:1024][:, 128 * c:128 * (c + 1)],
                                            w["ident"][:])
                        va = qkv.tile([128, 8, 17], BF16, tag=f"vaug{c}")
                        src = vtp[:].rearrange("p (x h j) -> p (x h) j", x=2, h=4)
                        nc.vector.tensor_copy(va[:, :, 0:16], src[:, :, 0:16])
                        nc.vector.memset(va[:, :, 16:17], 1.0)
                        vaug.append(va)

                    oX = []
                    for _oi in range(2):
                        oXt = ps_o.tile([128, 512], F32, tag="o", name=f"oX{_oi}")
                        oX.append(oXt)
                    for h in range(HEADS):
                        X = 0 if h < 4 else 1
                        hh = h % 4
                        Asb = att.tile([128, 2048], BF16, tag="Asb")
                        for pair in range(2):
                            stile = ps_s.tile([128, 1024], F32, tag="s")
                            for k in range(2):
                                c = 2 * pair + k
                                nc.tensor.matmul(
                                    stile[:, 512 * k:512 * (k + 1)],
                                    kT[:, 512 * X + 128 * c:512 * X + 128 * (c + 1)],
                                    qTm[h][:], start=True, stop=True)
                            nc.scalar.activation(
                                Asb[:, 1024 * pair:1024 * (pair + 1)], stile[:],
                                AF.Exp, scale=0.25)
                        if masked:
                            nc.vector.tensor_tensor(Asb[:], Asb[:], adj[:], OP.mult)
                        for c in range(4):
                            nc.tensor.matmul(
                                oX[X][32 * hh:32 * hh + 17, :],
                                vaug[c][:, h, :],
                                Asb[:, 512 * c:512 * (c + 1)],
                                start=(c == 0), stop=(c == 3),
                                tile_position=(0, 32 * hh))

                    # recipZ + o-normalization + Wo
                    owp = ps_m1.tile([128, 512], F32, tag="m1")
                    for X in range(2):
                        zs = wk.tile([128, 512], F32, tag="zs")
                        nc.vector.tensor_scalar_max(zs[:], oX[X][:], 1e-20)
                        lz = wk.tile([128, 512], F32, tag="lz")
                        nc.scalar.activation(lz[:], zs[:], AF.Ln)
                        R = wk.tile([128, 512], BF16, tag="R")
                        nc.scalar.activation(R[:], lz[:], AF.Exp, scale=-1.0)
                        rbp = ps_m2.tile([128, 512], F32, tag="m2")
                        nc.tensor.matmul(rbp[:], w["RselA" if X == 0 else "RselB"][:],
                                         R[:], start=True, stop=True)
                        Rb = wk.tile([128, 512], F32, tag="Rb")
                        nc.vector.tensor_copy(Rb[:], rbp[:])
                        oT = wk.tile([128, 512], BF16, tag=f"oT{X}")
                        nc.vector.tensor_tensor(oT[:], oX[X][:], Rb[:], OP.mult)
                        nc.tensor.matmul(owp[:], w[("WoA" if X == 0 else "WoB") + str(li)][:],
                                         oT[:], start=(X == 0), stop=(X == 1))

                    u = wk.tile([128, 512], F32, tag="u")
                    nc.vector.tensor_tensor(u[:], owp[:], hd[:], OP.add)

                    # LN1
                    ucp = ps_m1.tile([128, 512], F32, tag="m1")
                    nc.tensor.matmul(ucp[:], w["CEN"][:], u[:], start=True, stop=True)
                    uc = evac(wk, ucp[:], [128, 512], F32, tag="uc")
                    sq = wk.tile([128, 512], BF16, tag="sq")
                    nc.vector.tensor_tensor(sq[:], uc[:], uc[:], OP.mult)
                    ssp = ps_m1.tile([128, 512], F32, tag="m1")
                    nc.tensor.matmul(ssp[:], w["onesmat"][:], sq[:],
                                     start=True, stop=True)
                    vb = wk.tile([128, 512], F32, tag="vb")
                    nc.vector.tensor_scalar(vb[:], ssp[:], 1.0 / 128.0, EPS,
                                            OP.mult, OP.add)
                    lnv = wk.tile([128, 512], F32, tag="lnv")
                    nc.scalar.activation(lnv[:], vb[:], AF.Ln)
                    rb2 = wk.tile([128, 512], F32, tag="rb2")
                    nc.scalar.activation(rb2[:], lnv[:], AF.Exp, scale=-0.5)
                    t2 = wk.tile([128, 512], F32, tag="t2")
                    nc.vector.tensor_tensor(t2[:], uc[:], rb2[:], OP.mult)
                    hd1 = wk.tile([128, 512], F32, tag="hd1")
                    nc.scalar.activation(hd1[:], t2[:], AF.Identity,
                                         scale=w[f"ln1g{li}"][:, 0:1],
                                         bias=w[f"ln1b{li}"][:, 0:1])
                    hd1_bf = wk.tile([128, 512], BF16, tag="hd1_bf")
                    nc.vector.tensor_copy(hd1_bf[:], hd1[:])

                    # FFN
                    f1p = ps_m1.tile([128, 512], F32, tag="m1")
                    nc.tensor.matmul(f1p[:], w[f"fW1{li}"][:], hd1_bf[:],
                                     start=True, stop=True)
                    f1 = wk.tile([128, 512], BF16, tag="f1")
                    nc.scalar.activation(f1[:], f1p[:], AF.Relu,
                                         bias=w[f"fb1{li}"][:, 0:1])
                    f2p = ps_m1.tile([128, 512], F32, tag="m1")
                    nc.tensor.matmul(f2p[:], w[f"fW2{li}"][:], f1[:],
                                     start=True, stop=True)
                    f2b = wk.tile([128, 512], F32, tag="f2b")
                    nc.scalar.activation(f2b[:], f2p[:], AF.Identity,
                                         bias=w[f"fb2{li}"][:, 0:1])
                    u2 = wk.tile([128, 512], F32, tag="u2")
                    nc.vector.tensor_tensor(u2[:], f2b[:], hd1[:], OP.add)

                    # LN2
                    ucp2 = ps_m1.tile([128, 512], F32, tag="m1")
                    nc.tensor.matmul(ucp2[:], w["CEN"][:], u2[:], start=True, stop=True)
                    uc2 = evac(wk, ucp2[:], [128, 512], F32, tag="uc")
                    sq2 = wk.tile([128, 512], BF16, tag="sq")
                    nc.vector.tensor_tensor(sq2[:], uc2[:], uc2[:], OP.mult)
                    ssp2 = ps_m1.tile([128, 512], F32, tag="m1")
                    nc.tensor.matmul(ssp2[:], w["onesmat"][:], sq2[:],
                                     start=True, stop=True)
                    vb2 = wk.tile([128, 512], F32, tag="vb")
                    nc.vector.tensor_scalar(vb2[:], ssp2[:], 1.0 / 128.0, EPS,
                                            OP.mult, OP.add)
                    lnv2 = wk.tile([128, 512], F32, tag="lnv")
                    nc.scalar.activation(lnv2[:], vb2[:], AF.Ln)
                    rb3 = wk.tile([128, 512], F32, tag="rb2")
                    nc.scalar.activation(rb3[:], lnv2[:], AF.Exp, scale=-0.5)
                    t3 = wk.tile([128, 512], F32, tag="t2")
                    nc.vector.tensor_tensor(t3[:], uc2[:], rb3[:], OP.mult)
                    hd = wk.tile([128, 512], F32, tag="hd")
                    nc.scalar.activation(hd[:], t3[:], AF.Identity,
                                         scale=w[f"ln2g{li}"][:, 0:1],
                                         bias=w[f"ln2b{li}"][:, 0:1])
                    hd_bf = wk.tile([128, 512], BF16, tag="hd_bf")
                    nc.vector.tensor_copy(hd_bf[:], hd[:])

                # ---- PMA pooling ----
                kpp = ps_m1.tile([128, 512], F32, tag="m1")
                nc.tensor.matmul(kpp[:], w["pWk"][:], hd_bf[:], start=True, stop=True)
                kpb = wk.tile([128, 512], BF16, tag="kpb")
                nc.vector.tensor_copy(kpb[:], kpp[:])
                vpp = ps_m1.tile([128, 512], F32, tag="m1")
                nc.tensor.matmul(vpp[:], w["pWv"][:], hd_bf[:], start=True, stop=True)
                vpb = wk.tile([128, 512], BF16, tag="vpb")
                nc.vector.tensor_copy(vpb[:], vpp[:])

                spp = ps_m1.tile([128, 512], F32, tag="m1")
                nc.tensor.matmul(spp[0:8, :], w["qbd"][:], kpb[:],
                                 start=True, stop=True)
                esb = wk.tile([8, 512], F32, tag="esb")
                nc.scalar.activation(esb[:], spp[0:8, :], AF.Exp, scale=0.25)
                zp = wk.tile([8, 1], F32, tag="zp")
                nc.vector.tensor_reduce(zp[:], esb[:], mybir.AxisListType.X, OP.add)
                rz = wk.tile([8, 1], F32, tag="rz")
                nc.vector.reciprocal(rz[:], zp[:])
                asb = wk.tile([128, 512], BF16, tag="asb")
                nc.vector.memset(asb[:], 0.0)
                nc.vector.tensor_tensor(asb[0:8, :], esb[:],
                                        rz[:].to_broadcast((8, 512)), OP.mult)

                # transpose a [8,512] (padded to 128 rows) and Vp
                atp = ps_m2.tile([128, 512], BF16, tag="m2")
                for c in range(4):
                    nc.tensor.transpose(atp[:, 128 * c:128 * (c + 1)],
                                        asb[:, 128 * c:128 * (c + 1)],
                                        w["ident"][:])
                aT = wk.tile([128, 512], BF16, tag="aT")
                nc.vector.tensor_copy(aT[:], atp[:])
                vtp2 = ps_m2.tile([128, 512], BF16, tag="m2")
                for c in range(4):
                    nc.tensor.transpose(vtp2[:, 128 * c:128 * (c + 1)],
                                        vpb[:, 128 * c:128 * (c + 1)],
                                        w["ident"][:])
                Vp = wk.tile([128, 512], BF16, tag="Vp")
                nc.vector.tensor_copy(Vp[:], vtp2[:])

                opX = []
                for _oi in range(2):
                    opXt = ps_o.tile([128, 512], F32, tag="o", name=f"opX{_oi}")
                    opX.append(opXt)
                for h in range(HEADS):
                    X = 0 if h < 4 else 1
                    hh = h % 4
                    for c in range(4):
                        nc.tensor.matmul(
                            opX[X][32 * hh:32 * hh + 16, 0:1],
                            Vp[:, 128 * c + 16 * h:128 * c + 16 * h + 16],
                            aT[:, 128 * c + h:128 * c + h + 1],
                            start=(c == 0), stop=(c == 3),
                            tile_position=(0, 32 * hh))
                opsb = [None, None]
                for X in range(2):
                    t = wk.tile([128, 16], BF16, tag=f"opsb{X}")
                    nc.vector.tensor_copy(t[:], opX[X][:, 0:16])
                    opsb[X] = t
                plp = ps_m1.tile([128, 512], F32, tag="m1")
                nc.tensor.matmul(plp[:, 0:1], w["pWoA"][:], opsb[0][:, 0:1],
                                 start=True, stop=False)
                nc.tensor.matmul(plp[:, 0:1], w["pWoB"][:], opsb[1][:, 0:1],
                                 start=False, stop=True)
                nc.vector.tensor_copy(pooled_acc[:, g:g + 1], plp[:, 0:1])

            # ---- output MLP over all graphs of this core ----
            t1p = ps_m1.tile([128, 512], F32, tag="m1")
            nc.tensor.matmul(t1p[:, 0:gpc], w["outW1"][:], pooled_acc[:],
                             start=True, stop=True)
            t1 = wk.tile([128, gpc], F32, tag="t1")
            nc.scalar.activation(t1[:], t1p[:, 0:gpc], AF.Relu,
                                 bias=w["outb1"][:, 0:1])
            t2p = ps_m1.tile([128, 512], F32, tag="m1")
            nc.tensor.matmul(t2p[0:1, 0:gpc], w["outW2"][:], t1[:],
                             start=True, stop=True)
            outsb = wk.tile([1, gpc], F32, tag="outsb")
            nc.vector.tensor_copy(outsb[:], t2p[0:1, 0:gpc])
            nc.sync.dma_start(d_out[:], outsb[:])

    nc.compile()
    return nc


_CACHED = {}


def _get_program(gpc):
    if gpc not in _CACHED:
        _CACHED[gpc] = _build_program(gpc)
    return _CACHED[gpc]


def _prep_weights(kw):
    """Host-side packing of weights into the layouts the device expects."""
    f = np.asarray
    out = {}
    in_W1 = f(kw["in_W1"], np.float32)  # [160, 128]
    W1a = np.zeros((128, 128), np.float32); W1a[:64] = in_W1[0:64]
    W1b = np.zeros((128, 128), np.float32); W1b[:64] = in_W1[64:128]
    W1c = np.zeros((128, 128), np.float32); W1c[:32] = in_W1[128:160]
    out["W1a"], out["W1b"], out["W1c"] = W1a, W1b, W1c
    out["inb1"] = f(kw["in_b1"], np.float32).reshape(128, 1)
    out["inb2"] = f(kw["in_b2"], np.float32).reshape(128, 1)
    out["inW2"] = f(kw["in_W2"], np.float32).astype(_BF)
    out["CEN"] = (np.eye(128) - np.ones((128, 128)) / 128.0).astype(np.float32)
    out["onesmat"] = np.ones((128, 128), _BF)
    out["ident"] = np.eye(128, dtype=_BF)
    # Rsel: Rbig rows 32h+j (j<16) <- R row 32h+16
    RselA = np.zeros((128, 128), np.float32)
    for hh in range(4):
        RselA[32 * hh + 16, 32 * hh:32 * hh + 16] = 1.0
    out["RselA"] = RselA.astype(_BF)
    out["RselB"] = RselA.astype(_BF)

    def pad_heads(W):
        # W [128, 128] -> (A, B): A cols 32hh+j = W[:, (hh)*16+j], h<4
        A = np.zeros((128, 128), np.float32)
        Bm = np.zeros((128, 128), np.float32)
        for hh in range(4):
            A[:, 32 * hh:32 * hh + 16] = W[:, 16 * hh:16 * hh + 16]
            Bm[:, 32 * hh:32 * hh + 16] = W[:, 16 * (hh + 4):16 * (hh + 4) + 16]
        return A.astype(_BF), Bm.astype(_BF)

    def pad_heads_rows(W):
        # W [128, 128] -> (A, B) with PADDED ROWS: A rows 32hh+j = W[16hh+j,:]
        A = np.zeros((128, 128), np.float32)
        Bm = np.zeros((128, 128), np.float32)
        for hh in range(4):
            A[32 * hh:32 * hh + 16, :] = W[16 * hh:16 * hh + 16, :]
            Bm[32 * hh:32 * hh + 16, :] = W[16 * (hh + 4):16 * (hh + 4) + 16, :]
        return A.astype(_BF), Bm.astype(_BF)

    Wq, Wk, Wv, Wo = (f(kw[n], np.float32) for n in ("Wq", "Wk", "Wv", "Wo"))
    for i in range(L):
        out[f"WqA{i}"], out[f"WqB{i}"] = pad_heads(Wq[i])
        out[f"WkA{i}"], out[f"WkB{i}"] = pad_heads(Wk[i])
        out[f"WvA{i}"], out[f"WvB{i}"] = pad_heads(Wv[i])
        out[f"WoA{i}"], out[f"WoB{i}"] = pad_heads_rows(Wo[i])
        out[f"fW1{i}"] = f(kw["f_W1"][i], np.float32).astype(_BF)
        out[f"fW2{i}"] = f(kw["f_W2"][i], np.float32).astype(_BF)
        out[f"fb1{i}"] = f(kw["f_b1"][i], np.float32).reshape(128, 1)
        out[f"fb2{i}"] = f(kw["f_b2"][i], np.float32).reshape(128, 1)
        out[f"ln1g{i}"] = f(kw["ln1_g"][i], np.float32).reshape(128, 1)
        out[f"ln1b{i}"] = f(kw["ln1_b"][i], np.float32).reshape(128, 1)
        out[f"ln2g{i}"] = f(kw["ln2_g"][i], np.float32).reshape(128, 1)
        out[f"ln2b{i}"] = f(kw["ln2_b"][i], np.float32).reshape(128, 1)

    # PMA
    seed = f(kw["seed"], np.float32)
    qp = seed @ f(kw["p_Wq"], np.float32)  # [128]
    qbd = np.zeros((128, 8), np.float32)
    for h in range(HEADS):
        qbd[16 * h:16 * h + 16, h] = qp[16 * h:16 * h + 16]
    out["qbd"] = qbd.astype(_BF)
    out["pWk"] = f(kw["p_Wk"], np.float32).astype(_BF)
    out["pWv"] = f(kw["p_Wv"], np.float32).astype(_BF)
    pWo = f(kw["p_Wo"], np.float32)
    # pWoA rows 32hh+j <-> p_Wo[(hh)*16+j, :] (heads 0-3); B heads 4-7
    A = np.zeros((128, 128), np.float32)
    Bm = np.zeros((128, 128), np.float32)
    for hh in range(4):
        A[32 * hh:32 * hh + 16, :] = pWo[16 * hh:16 * hh + 16, :]
        Bm[32 * hh:32 * hh + 16, :] = pWo[16 * (hh + 4):16 * (hh + 4) + 16, :]
    out["pWoA"] = A.astype(_BF)
    out["pWoB"] = Bm.astype(_BF)
    out["outW1"] = f(kw["out_W1"], np.float32)
    out["outb1"] = f(kw["out_b1"], np.float32).reshape(128, 1)
    out["outW2"] = f(kw["out_W2"], np.float32).reshape(128, 1)
    return out, float(np.asarray(kw["out_b2"]).reshape(-1)[0])


def _reference_numpy(kw):
    """Exact numpy fallback for unstructured inputs."""
    x = np.asarray(kw["x"], np.float32)
    ea = np.asarray(kw["edge_attr"], np.float32)
    src = np.asarray(kw["edge_index"])[0].astype(np.int64)
    dst = np.asarray(kw["edge_index"])[1].astype(np.int64)
    nb = np.asarray(kw["node_batch"]).astype(np.int64)
    ME = int(np.asarray(kw["max_edges"]))
    E = src.shape[0]

    def ln(x_, g_, b_):
        m = x_.mean(-1, keepdims=True)
        v = x_.var(-1, keepdims=True)
        return (x_ - m) / np.sqrt(v + 1e-5) * g_ + b_

    def mha(q_in, kv_in, Wq_, Wk_, Wv_, Wo_, mask):
        Bq, Lq, _ = q_in.shape
        Lk = kv_in.shape[1]
        q = (q_in @ Wq_).reshape(Bq, Lq, HEADS, DH)
        k = (kv_in @ Wk_).reshape(Bq, Lk, HEADS, DH)
        v = (kv_in @ Wv_).reshape(Bq, Lk, HEADS, DH)
        s = np.einsum('bqhd,bkhd->bhqk', q, k) / np.sqrt(DH)
        s = np.where(mask[:, None, :, :], s, -1e9).astype(np.float32)
        s = s - s.max(-1, keepdims=True)
        a = np.exp(s); a /= a.sum(-1, keepdims=True)
        o = np.einsum('bhqk,bkhd->bqhd', a, v).reshape(Bq, Lq, HID)
        return o @ Wo_

    feat = np.concatenate([x[src], x[dst], ea], 1)
    h = np.maximum(feat @ kw["in_W1"] + kw["in_b1"], 0) @ kw["in_W2"] + kw["in_b2"]
    eb = nb[src]
    counts = np.bincount(eb, minlength=B)
    offsets = np.concatenate([[0], np.cumsum(counts)[:-1]])
    pos = np.arange(E) - offsets[eb]
    dense_h = np.zeros((B, ME, HID), np.float32)
    dense_h[eb, pos] = h
    pad = np.zeros((B, ME), bool); pad[eb, pos] = True
    ds_ = np.full((B, ME), -1, np.int64); ds_[eb, pos] = src
    dd_ = np.full((B, ME), -2, np.int64); dd_[eb, pos] = dst
    adj = ((ds_[:, :, None] == ds_[:, None, :]) | (ds_[:, :, None] == dd_[:, None, :]) |
           (dd_[:, :, None] == ds_[:, None, :]) | (dd_[:, :, None] == dd_[:, None, :]))
    eye = np.eye(ME, dtype=bool)[None]
    adj_mask = (adj & pad[:, None, :]) | eye
    pad2 = np.broadcast_to(pad[:, None, :], (B, ME, ME)) | eye
    hd = dense_h
    for i, t in enumerate(LAYER_TYPES):
        m = adj_mask if t == 'M' else pad2
        a = mha(hd, hd, kw["Wq"][i], kw["Wk"][i], kw["Wv"][i], kw["Wo"][i], m)
        hd = ln(hd + a, kw["ln1_g"][i], kw["ln1_b"][i])
        fv = np.maximum(hd @ kw["f_W1"][i] + kw["f_b1"][i], 0) @ kw["f_W2"][i] + kw["f_b2"][i]
        hd = ln(hd + fv, kw["ln2_g"][i], kw["ln2_b"][i])
    q = np.broadcast_to(np.asarray(kw["seed"], np.float32)[None, None, :], (B, 1, HID))
    pooled = mha(q, hd, kw["p_Wq"], kw["p_Wk"], kw["p_Wv"], kw["p_Wo"],
                 pad[:, None, :])[:, 0]
    logits = np.maximum(pooled @ kw["out_W1"] + kw["out_b1"], 0) @ kw["out_W2"] + kw["out_b2"]
    return logits.reshape(-1).astype(np.float32)


def kernel(**inputs):
    kw = {k: np.asarray(v) for k, v in inputs.items()}
    src = kw["edge_index"][0].astype(np.int64)
    dst = kw["edge_index"][1].astype(np.int64)
    nb = kw["node_batch"].astype(np.int64)
    ME = int(np.asarray(kw["max_edges"]))

    # structured-input check: 128 nodes/graph sorted, 512 edges/graph in order
    structured = (
        ME == EPG
        and nb.shape[0] == B * NPG
        and np.array_equal(nb, np.repeat(np.arange(B), NPG))
        and src.shape[0] == B * EPG
        and np.array_equal(src // NPG, np.repeat(np.arange(B), EPG))
        and np.array_equal(dst // NPG, np.repeat(np.arange(B), EPG))
    )
    if not structured:
        return _reference_numpy(kw)

    x = kw["x"].astype(np.float32)          # [8192, 64]
    ea = kw["edge_attr"].astype(np.float32)  # [32768, 32]
    srcl = (src % NPG).reshape(B, EPG)
    dstl = (dst % NPG).reshape(B, EPG)

    # host layout prep
    xT = np.ascontiguousarray(
        x.reshape(B, NPG, NODE_DIM).transpose(0, 2, 1)).astype(np.float32)
    eaT = np.ascontiguousarray(
        ea.reshape(B, EPG, EDGE_DIM).transpose(0, 2, 1)).astype(np.float32)
    srcb = srcl.astype(np.float32).reshape(B, 1, EPG)
    dstb = dstl.astype(np.float32).reshape(B, 1, EPG)

    wmap, out_b2 = _prep_weights(kw)
    wmap["iota"] = np.arange(128, dtype=np.float32).reshape(128, 1)
    nc = _get_program(GPC)

    in_maps = []
    for c in range(N_CORES):
        sl = slice(c * GPC, (c + 1) * GPC)
        m = dict(xT=np.ascontiguousarray(xT[sl]),
                 eaT=np.ascontiguousarray(eaT[sl]),
                 srcb=np.ascontiguousarray(srcb[sl]),
                 dstb=np.ascontiguousarray(dstb[sl]))
        for k2, v2 in wmap.items():
            m[k2] = v2
        in_maps.append(m)

    res = run_bass_kernel_spmd(nc, in_maps, list(range(N_CORES)))
    outs = [np.asarray(res.results[c]["out"], np.float32).reshape(-1)
            for c in range(N_CORES)]
    logits = np.concatenate(outs) + out_b2
    return logits.astype(np.float32)


if __name__ == "__main__":
    pass
